# revision 36
# baseline (speedup 1.0000x reference)
"""Banded (Luong) attention TRN2 Bass kernel, 8-core SPMD, bf16 compute.

Problem: h [4, 4096, 1024] f32, W [1024, 1024] f32, T_hist=256.
  K = h @ W.T ; scores = (h @ K^T) / sqrt(H) with causal band
  (q - 255 <= k <= q); out = softmax(scores) @ h.

Sharding: data-parallel over batch (4) x sequence halves (2) -> 8 cores,
no cross-core communication. Each core handles 2048 queries; its key
region is 18 blocks of 128 (2 lead blocks for the band history,
zero-padded for the first half of each sequence).

Per-core algorithm (all matmuls bf16, fp32 PSUM accumulate; inputs are
converted to bf16 on the host, halving HBM traffic):
  warmup  dummy matmuls on zeroed SBUF hold the PE HAM clock gate at
          2.4 GHz while the first input DMAs land.
  proj    Q'T[m, q] = (h_q @ W)^T for all 2048 queries, 8-matmul
          accumulation groups of <=512 cols into 4-deep rotating PSUM,
          copied to SBUF bf16 on alternating DVE/ACT.
  ST      key-major: key block R scores against the 384 contiguous
          queries (q blocks R-2..R) that attend it -- one 8-matmul
          ap<=384 accumulation per R, LDWEIGHTS fully hidden. One
          shared additive [causal|full|upper] mask (DVE), exp with
          fused 1/32 scale (ACT) -> pt bf16.
  ctx     per query block: 6 matmuls of 512 cols accumulate P^T @ h;
          softmax denominators via DVE pre-sum of the 3 band slices +
          one ones-matmul into the st tile's spare column; normalize
          with per-partition 1/sum during the PSUM->SBUF copy
          (ACT/DVE split); DMA out per 512-col half.
  ctx_block(R-4) is issued after ST(R) so every softmax chain hides
  under PE work, keeping the matmul stream dense (no HAM re-throttle).
"""

import ml_dtypes
import numpy as np

import concourse.bass as bass
import concourse.mybir as mybir
import concourse.tile as tile
from concourse import bacc

B, T, H = 4, 4096, 1024
T_HIST = 256
N_CORES = 8
QB = 16            # 128-row query blocks per core
RB = QB + 2        # key-region blocks per core (2 lead blocks)
QPC = 2048         # queries per core
BF16 = mybir.dt.bfloat16
F32 = mybir.dt.float32
NEG = np.float32(-1e9)
INV_SQRT_H = 1.0 / 32.0
NPBF16 = ml_dtypes.bfloat16

_CACHE = {}


def _kernel_body(tc, out, hTr, haug, Wr, masks, onesd):
    nc = tc.nc

    with (
        tc.tile_pool(name="singles", bufs=1) as singles,
        tc.tile_pool(name="pt", bufs=6) as pt_pool,
        tc.tile_pool(name="ptsum", bufs=3) as ptsum_pool,
        tc.tile_pool(name="ctxs", bufs=3) as ctxs_pool,
        tc.tile_pool(name="recip", bufs=4) as recip_pool,
        tc.tile_pool(name="ps", bufs=4, space="PSUM") as ps_pool,
        tc.tile_pool(name="st", bufs=4, space="PSUM") as st_pool,
    ):
        # --- PE warmup: dummy matmuls on zeroed SBUF keep the HAM
        # activity window busy while the first input DMAs land ---
        warm = singles.tile([128, 256], BF16)
        nc.vector.memset(warm[:], 0.0)
        for w in range(60):
            wps = ps_pool.tile([128, 128], F32, tag="ps")
            nc.tensor.matmul(
                wps[:], warm[:, 0:128], warm[:, 128:256], start=True, stop=True
            )

        # --- resident inputs, DMA'd in exact consumption order: the W
        # slices feed proj groups mc=0..7 of tt=0 progressively. The
        # first hT pieces go on the ACT sequencer so their issue overlaps
        # the W issues on SYNC; masks/ha go on DVE (needed much later) ---
        W_sb = singles.tile([128, 8, H], BF16)            # 2.10 MiB
        hT_sb = singles.tile([128, 8, RB * 128], BF16)    # 4.72 MiB
        ha_sb = singles.tile([128, RB, H], BF16)          # 4.72 MiB
        qt = singles.tile([128, 8, QPC], BF16)            # 4.19 MiB
        mask_sb = singles.tile([128, 2, 384], F32)
        ones_sb = singles.tile([128, 1], BF16)

        nc.scalar.dma_start(hT_sb[:, :, 256:512], hTr[:, :, 256:512])
        nc.scalar.dma_start(hT_sb[:, :, 512:768], hTr[:, :, 512:768])
        for mc in range(8):
            nc.sync.dma_start(
                W_sb[:, :, mc * 128 : (mc + 1) * 128],
                Wr[:, :, mc * 128 : (mc + 1) * 128],
            )
        nc.sync.dma_start(hT_sb[:, :, 768:1280], hTr[:, :, 768:1280])
        nc.sync.dma_start(hT_sb[:, :, 1280:2304], hTr[:, :, 1280:2304])
        nc.sync.dma_start(hT_sb[:, :, 0:256], hTr[:, :, 0:256])
        nc.sync.dma_start(mask_sb[:], masks[:])
        nc.sync.dma_start(ones_sb[:], onesd[:])
        nc.sync.dma_start(ha_sb[:], haug[:])

        # --- projection: Q'T[m, q] for all 2048 queries. The first four
        # groups run at 256 cols so the PE can start on 0.76 MiB of input ---
        def proj_group(mc, q0, cw):
            ps = ps_pool.tile([128, cw], F32, tag="ps")
            for oc in range(8):
                nc.tensor.matmul(
                    ps[:],
                    W_sb[:, oc, mc * 128 : (mc + 1) * 128],
                    hT_sb[:, oc, 256 + q0 : 256 + q0 + cw],
                    start=(oc == 0),
                    stop=(oc == 7),
                )
            if mc % 2 == 0:
                nc.vector.tensor_copy(qt[:, mc, q0 : q0 + cw], ps[:])
            else:
                nc.scalar.copy(qt[:, mc, q0 : q0 + cw], ps[:])

        proj_group(0, 0, 256)
        proj_group(1, 0, 256)
        proj_group(0, 256, 256)
        proj_group(1, 256, 256)
        for mc in range(2, 8):
            proj_group(mc, 0, 512)
        for tt in range(1, 4):
            for mc in range(8):
                proj_group(mc, tt * 512, 512)

        # --- attention, key-major: key block R scores against the 384
        # contiguous queries (q blocks R-2..R) that attend it, one 8-matmul
        # accumulation of ap<=384 per R (LDWEIGHTS fully hidden). Column c
        # of st_R belongs to q block i = R-2 + c//128 at band position
        # j = R - i, so the additive mask is the same [caus|full|su]
        # pattern for every interior R. ---
        st_tiles = {}   # per key block R
        pt_tiles = {}

        def ST(R):
            st = st_pool.tile([128, 388], F32, tag="st")
            st_tiles[R] = st
            lo = max(0, 2 - R) * 128
            hi = (min(15, R) - R + 3) * 128
            qlo = (R - 2) * 128 + lo
            for mc in range(8):
                nc.tensor.matmul(
                    st[:, lo:hi],
                    hT_sb[:, mc, R * 128 : (R + 1) * 128],
                    qt[:, mc, qlo : qlo + hi - lo],
                    start=(mc == 0), stop=(mc == 7),
                )
            msel = 1 if R <= 1 else 0    # lead blocks get the boundary mask
            nc.vector.tensor_add(
                st[:, lo:hi], st[:, lo:hi], mask_sb[:, msel, lo:hi]
            )
            pt = pt_pool.tile([128, 384], BF16, tag="pt")
            nc.scalar.activation(
                pt[:, lo:hi], st[:, lo:hi], mybir.ActivationFunctionType.Exp,
                scale=INV_SQRT_H,
            )
            pt_tiles[R] = pt

        def ctx_block(i):
            # band slice j of q block i lives in pt_{i+j} cols (2-j)*128
            sl = [
                pt_tiles[i + j][:, (2 - j) * 128 : (3 - j) * 128]
                for j in range(3)
            ]
            psm = ptsum_pool.tile([128, 128], BF16, tag="ptsum")
            nc.vector.tensor_add(psm[:], sl[0], sl[1])
            nc.vector.tensor_add(psm[:], psm[:], sl[2])
            sums = st_tiles[i + 2]
            nc.tensor.matmul(
                sums[:, 384:385], psm[:], ones_sb[:], start=True, stop=True
            )
            ca = ps_pool.tile([128, 512], F32, tag="ps")
            cb = ps_pool.tile([128, 512], F32, tag="ps")
            for j in range(3):
                nc.tensor.matmul(
                    ca[:], sl[j], ha_sb[:, i + j, 0:512],
                    start=(j == 0), stop=(j == 2),
                )
                nc.tensor.matmul(
                    cb[:], sl[j], ha_sb[:, i + j, 512:1024],
                    start=(j == 0), stop=(j == 2),
                )
            recip = recip_pool.tile([128, 1], F32)
            nc.vector.reciprocal(recip[:], sums[:, 384:385])
            ctxs = ctxs_pool.tile([128, 1024], F32)
            nc.scalar.mul(ctxs[:, 0:512], ca[:], mul=recip[:])
            nc.sync.dma_start(out[i][:, 0:512], ctxs[:, 0:512])
            nc.vector.tensor_scalar_mul(ctxs[:, 512:1024], cb[:], recip[:])
            nc.sync.dma_start(out[i][:, 512:1024], ctxs[:, 512:1024])

        # Steady-state skew is 4 (ctx of block i after ST(i+4)) so every
        # softmax chain hides under ~2.8us of PE work. The last iterations
        # tighten to skew 3 -- their exp inputs are already resident -- so
        # only ctx_block(15) trails the final ST instead of three blocks.
        for R in range(RB):
            ST(R)
            if R == 16:
                ctx_block(12)
                ctx_block(13)
            elif R == 17:
                ctx_block(14)
            elif R >= 4:
                ctx_block(R - 4)
        ctx_block(15)


def _build():
    if "nc" in _CACHE:
        return _CACHE["nc"]
    nc = bacc.Bacc(
        "TRN2", target_bir_lowering=False, debug=False, num_devices=N_CORES
    )
    hTr = nc.dram_tensor("hTr", [128, 8, RB * 128], BF16, kind="ExternalInput").ap()
    haug = nc.dram_tensor("haug", [128, RB, H], BF16, kind="ExternalInput").ap()
    Wr = nc.dram_tensor("Wr", [128, 8, H], BF16, kind="ExternalInput").ap()
    masks = nc.dram_tensor("masks", [128, 2, 384], F32, kind="ExternalInput").ap()
    onesd = nc.dram_tensor("onesd", [128, 1], BF16, kind="ExternalInput").ap()
    out = nc.dram_tensor("out", [QB, 128, H], F32, kind="ExternalOutput").ap()
    with tile.TileContext(nc) as tc:
        _kernel_body(tc, out, hTr, haug, Wr, masks, onesd)
    nc.compile()
    _CACHE["nc"] = nc
    return nc


def _host_masks():
    # column c of an st_R tile: band position j = 2 - c//128
    # j=2 -> causal (kk <= qi), j=1 -> full, j=0 -> strict upper (kk > qi)
    kk = np.arange(128, dtype=np.int64)[:, None]
    qi = np.arange(128, dtype=np.int64)[None, :]
    su = np.where(kk > qi, np.float32(0.0), NEG).astype(np.float32)
    caus = np.where(kk <= qi, np.float32(0.0), NEG).astype(np.float32)
    interior = np.empty((128, 384), np.float32)
    interior[:, 0:128] = caus
    interior[:, 128:256] = 0.0
    interior[:, 256:384] = su
    boundary = np.full((128, 384), NEG, np.float32)
    return interior, boundary


def _prepare_in_maps(h, W):
    interior, boundary = _host_masks()
    masks_b = np.ascontiguousarray(np.stack([interior, boundary], axis=1))
    masks_i = np.ascontiguousarray(np.stack([interior, interior], axis=1))
    h16 = h.astype(NPBF16)
    W16 = W.astype(NPBF16)
    Wr = np.ascontiguousarray(W16.reshape(8, 128, H).transpose(1, 0, 2))
    ones = np.ones((128, 1), NPBF16)
    in_maps = []
    for core in range(N_CORES):
        b, half = core // 2, core % 2
        k_lo = half * QPC - 256            # region global key start
        pad = max(0, -k_lo)                # 256 for half 0, else 0
        k_lo = max(0, k_lo)
        k_hi = half * QPC + QPC

        # region keys-major [2304, 1024] bf16, zero lead pad
        region = np.zeros((RB * 128, H), NPBF16)
        region[pad:] = h16[b, k_lo:k_hi]

        # feature-major [128, 8, 2304]
        hTr = np.ascontiguousarray(
            region.T.reshape(8, 128, RB * 128).transpose(1, 0, 2)
        )
        # keys-major [128, 18, 1024]
        haug = np.ascontiguousarray(
            region.reshape(RB, 128, H).transpose(1, 0, 2)
        )
        in_maps.append(
            {
                "hTr": hTr,
                "haug": haug,
                "Wr": Wr,
                "masks": masks_b if half == 0 else masks_i,
                "onesd": ones,
            }
        )
    return in_maps


def _assemble(results):
    out = np.empty((B, T, H), np.float32)
    for core in range(N_CORES):
        b, half = core // 2, core % 2
        out[b, half * QPC : (half + 1) * QPC] = (
            results[core]["out"].reshape(QPC, H)
        )
    return out


def kernel(h, W, T_hist):
    h = np.asarray(h, dtype=np.float32)
    W = np.asarray(W, dtype=np.float32)
    assert int(T_hist) == T_HIST
    assert h.shape == (B, T, H) and W.shape == (H, H)

    from concourse.bass_utils import run_bass_kernel_spmd

    nc = _build()
    in_maps = _prepare_in_maps(h, W)
    res = run_bass_kernel_spmd(nc, in_maps, core_ids=list(range(N_CORES)))
    return _assemble(res.results)


# revision 37
# speedup vs baseline: 1.0169x; 1.0169x over previous
"""Banded (Luong) attention TRN2 Bass kernel, 8-core SPMD, bf16 compute.

Problem: h [4, 4096, 1024] f32, W [1024, 1024] f32, T_hist=256.
  K = h @ W.T ; scores = (h @ K^T) / sqrt(H) with causal band
  (q - 255 <= k <= q); out = softmax(scores) @ h.

Sharding: data-parallel over batch (4) x sequence halves (2) -> 8 cores,
no cross-core communication. Each core handles 2048 queries; its key
region is 18 blocks of 128 (2 lead blocks for the band history,
zero-padded for the first half of each sequence).

Per-core algorithm (all matmuls bf16, fp32 PSUM accumulate; inputs are
converted to bf16 on the host, halving HBM traffic):
  warmup  dummy matmuls on zeroed SBUF hold the PE HAM clock gate at
          2.4 GHz while the first input DMAs land.
  proj    Q'T[m, q] = (h_q @ W)^T for all 2048 queries, 8-matmul
          accumulation groups of <=512 cols into 4-deep rotating PSUM,
          copied to SBUF bf16 on alternating DVE/ACT.
  ST      key-major: key block R scores against the 384 contiguous
          queries (q blocks R-2..R) that attend it -- one 8-matmul
          ap<=384 accumulation per R, LDWEIGHTS fully hidden. One
          shared additive [causal|full|upper] mask (DVE), exp with
          fused 1/32 scale (ACT) -> pt bf16.
  ctx     per query block: 6 matmuls of 512 cols accumulate P^T @ h;
          softmax denominators via DVE pre-sum of the 3 band slices +
          one ones-matmul into the st tile's spare column; normalize
          with per-partition 1/sum during the PSUM->SBUF copy
          (ACT/DVE split); DMA out per 512-col half.
  ctx_block(R-4) is issued after ST(R) so every softmax chain hides
  under PE work, keeping the matmul stream dense (no HAM re-throttle).
"""

import ml_dtypes
import numpy as np

import concourse.bass as bass
import concourse.mybir as mybir
import concourse.tile as tile
from concourse import bacc

B, T, H = 4, 4096, 1024
T_HIST = 256
N_CORES = 8
QB = 16            # 128-row query blocks per core
RB = QB + 2        # key-region blocks per core (2 lead blocks)
QPC = 2048         # queries per core
BF16 = mybir.dt.bfloat16
F32 = mybir.dt.float32
NEG = np.float32(-1e9)
INV_SQRT_H = 1.0 / 32.0
NPBF16 = ml_dtypes.bfloat16

_CACHE = {}


def _kernel_body(tc, out, hTr, haug, Wr, masks, onesd):
    nc = tc.nc

    with (
        tc.tile_pool(name="singles", bufs=1) as singles,
        tc.tile_pool(name="pt", bufs=6) as pt_pool,
        tc.tile_pool(name="ptsum", bufs=3) as ptsum_pool,
        tc.tile_pool(name="ctxs", bufs=3) as ctxs_pool,
        tc.tile_pool(name="recip", bufs=4) as recip_pool,
        tc.tile_pool(name="ps", bufs=4, space="PSUM") as ps_pool,
        tc.tile_pool(name="st", bufs=4, space="PSUM") as st_pool,
    ):
        # --- PE warmup: dummy matmuls on zeroed SBUF keep the HAM
        # activity window busy while the first input DMAs land ---
        warm = singles.tile([128, 256], BF16)
        nc.vector.memset(warm[:], 0.0)
        for w in range(60):
            wps = ps_pool.tile([128, 128], F32, tag="ps")
            nc.tensor.matmul(
                wps[:], warm[:, 0:128], warm[:, 128:256], start=True, stop=True
            )

        # --- resident inputs, DMA'd in exact consumption order: the W
        # slices feed proj groups mc=0..7 of tt=0 progressively. The
        # first hT pieces go on the ACT sequencer so their issue overlaps
        # the W issues on SYNC; masks/ha go on DVE (needed much later) ---
        W_sb = singles.tile([128, 8, H], BF16)            # 2.10 MiB
        hT_sb = singles.tile([128, 8, RB * 128], BF16)    # 4.72 MiB
        ha_sb = singles.tile([128, RB, H], BF16)          # 4.72 MiB
        qt = singles.tile([128, 8, QPC], BF16)            # 4.19 MiB
        mask_sb = singles.tile([128, 2, 384], F32)
        ones_sb = singles.tile([128, 1], BF16)

        nc.scalar.dma_start(hT_sb[:, :, 256:512], hTr[:, :, 256:512])
        nc.scalar.dma_start(hT_sb[:, :, 512:768], hTr[:, :, 512:768])
        for mc in range(8):
            nc.sync.dma_start(
                W_sb[:, :, mc * 128 : (mc + 1) * 128],
                Wr[:, :, mc * 128 : (mc + 1) * 128],
            )
        nc.sync.dma_start(hT_sb[:, :, 768:1280], hTr[:, :, 768:1280])
        nc.sync.dma_start(hT_sb[:, :, 1280:2304], hTr[:, :, 1280:2304])
        nc.sync.dma_start(hT_sb[:, :, 0:256], hTr[:, :, 0:256])
        nc.sync.dma_start(mask_sb[:], masks[:])
        nc.sync.dma_start(ones_sb[:], onesd[:])
        nc.sync.dma_start(ha_sb[:], haug[:])

        # --- projection: Q'T[m, q] for all 2048 queries. The first four
        # groups run at 256 cols so the PE can start on 0.76 MiB of input ---
        def proj_group(mc, q0, cw):
            ps = ps_pool.tile([128, cw], F32, tag="ps")
            for oc in range(8):
                nc.tensor.matmul(
                    ps[:],
                    W_sb[:, oc, mc * 128 : (mc + 1) * 128],
                    hT_sb[:, oc, 256 + q0 : 256 + q0 + cw],
                    start=(oc == 0),
                    stop=(oc == 7),
                )
            if mc % 2 == 0:
                nc.vector.tensor_copy(qt[:, mc, q0 : q0 + cw], ps[:])
            else:
                nc.scalar.copy(qt[:, mc, q0 : q0 + cw], ps[:])

        proj_group(0, 0, 256)
        proj_group(1, 0, 256)
        proj_group(0, 256, 256)
        proj_group(1, 256, 256)
        for mc in range(2, 8):
            proj_group(mc, 0, 512)
        for tt in range(1, 4):
            for mc in range(8):
                proj_group(mc, tt * 512, 512)

        # --- attention, key-major: key block R scores against the 384
        # contiguous queries (q blocks R-2..R) that attend it, one 8-matmul
        # accumulation of ap<=384 per R (LDWEIGHTS fully hidden). Column c
        # of st_R belongs to q block i = R-2 + c//128 at band position
        # j = R - i, so the additive mask is the same [caus|full|su]
        # pattern for every interior R. ---
        st_tiles = {}   # per key block R
        pt_tiles = {}

        def ST(R):
            st = st_pool.tile([128, 388], F32, tag="st")
            st_tiles[R] = st
            lo = max(0, 2 - R) * 128
            hi = (min(15, R) - R + 3) * 128
            qlo = (R - 2) * 128 + lo
            for mc in range(8):
                nc.tensor.matmul(
                    st[:, lo:hi],
                    hT_sb[:, mc, R * 128 : (R + 1) * 128],
                    qt[:, mc, qlo : qlo + hi - lo],
                    start=(mc == 0), stop=(mc == 7),
                )
            msel = 1 if R <= 1 else 0    # lead blocks get the boundary mask
            nc.vector.tensor_add(
                st[:, lo:hi], st[:, lo:hi], mask_sb[:, msel, lo:hi]
            )
            pt = pt_pool.tile([128, 384], BF16, tag="pt")
            nc.scalar.activation(
                pt[:, lo:hi], st[:, lo:hi], mybir.ActivationFunctionType.Exp,
                scale=INV_SQRT_H,
            )
            pt_tiles[R] = pt

        def ctx_block(i):
            # band slice j of q block i lives in pt_{i+j} cols (2-j)*128
            sl = [
                pt_tiles[i + j][:, (2 - j) * 128 : (3 - j) * 128]
                for j in range(3)
            ]
            psm = ptsum_pool.tile([128, 128], BF16, tag="ptsum")
            nc.vector.tensor_add(psm[:], sl[0], sl[1])
            nc.vector.tensor_add(psm[:], psm[:], sl[2])
            sums = st_tiles[i + 2]
            nc.tensor.matmul(
                sums[:, 384:385], psm[:], ones_sb[:], start=True, stop=True
            )
            ca = ps_pool.tile([128, 512], F32, tag="ps")
            cb = ps_pool.tile([128, 512], F32, tag="ps")
            for j in range(3):
                nc.tensor.matmul(
                    ca[:], sl[j], ha_sb[:, i + j, 0:512],
                    start=(j == 0), stop=(j == 2),
                )
                nc.tensor.matmul(
                    cb[:], sl[j], ha_sb[:, i + j, 512:1024],
                    start=(j == 0), stop=(j == 2),
                )
            recip = recip_pool.tile([128, 1], F32)
            nc.vector.reciprocal(recip[:], sums[:, 384:385])
            ctxs = ctxs_pool.tile([128, 1024], F32)
            nc.scalar.mul(ctxs[:, 0:512], ca[:], mul=recip[:])
            nc.sync.dma_start(out[i][:, 0:512], ctxs[:, 0:512])
            nc.vector.tensor_scalar_mul(ctxs[:, 512:1024], cb[:], recip[:])
            nc.sync.dma_start(out[i][:, 512:1024], ctxs[:, 512:1024])

        for R in range(RB):
            ST(R)
            if R >= 4:
                ctx_block(R - 4)
        for i in range(14, 16):
            ctx_block(i)


def _build():
    if "nc" in _CACHE:
        return _CACHE["nc"]
    nc = bacc.Bacc(
        "TRN2", target_bir_lowering=False, debug=False, num_devices=N_CORES
    )
    hTr = nc.dram_tensor("hTr", [128, 8, RB * 128], BF16, kind="ExternalInput").ap()
    haug = nc.dram_tensor("haug", [128, RB, H], BF16, kind="ExternalInput").ap()
    Wr = nc.dram_tensor("Wr", [128, 8, H], BF16, kind="ExternalInput").ap()
    masks = nc.dram_tensor("masks", [128, 2, 384], F32, kind="ExternalInput").ap()
    onesd = nc.dram_tensor("onesd", [128, 1], BF16, kind="ExternalInput").ap()
    out = nc.dram_tensor("out", [QB, 128, H], F32, kind="ExternalOutput").ap()
    with tile.TileContext(nc) as tc:
        _kernel_body(tc, out, hTr, haug, Wr, masks, onesd)
    nc.compile()
    _CACHE["nc"] = nc
    return nc


def _host_masks():
    # column c of an st_R tile: band position j = 2 - c//128
    # j=2 -> causal (kk <= qi), j=1 -> full, j=0 -> strict upper (kk > qi)
    kk = np.arange(128, dtype=np.int64)[:, None]
    qi = np.arange(128, dtype=np.int64)[None, :]
    su = np.where(kk > qi, np.float32(0.0), NEG).astype(np.float32)
    caus = np.where(kk <= qi, np.float32(0.0), NEG).astype(np.float32)
    interior = np.empty((128, 384), np.float32)
    interior[:, 0:128] = caus
    interior[:, 128:256] = 0.0
    interior[:, 256:384] = su
    boundary = np.full((128, 384), NEG, np.float32)
    return interior, boundary


def _prepare_in_maps(h, W):
    interior, boundary = _host_masks()
    masks_b = np.ascontiguousarray(np.stack([interior, boundary], axis=1))
    masks_i = np.ascontiguousarray(np.stack([interior, interior], axis=1))
    h16 = h.astype(NPBF16)
    W16 = W.astype(NPBF16)
    Wr = np.ascontiguousarray(W16.reshape(8, 128, H).transpose(1, 0, 2))
    ones = np.ones((128, 1), NPBF16)
    in_maps = []
    for core in range(N_CORES):
        b, half = core // 2, core % 2
        k_lo = half * QPC - 256            # region global key start
        pad = max(0, -k_lo)                # 256 for half 0, else 0
        k_lo = max(0, k_lo)
        k_hi = half * QPC + QPC

        # region keys-major [2304, 1024] bf16, zero lead pad
        region = np.zeros((RB * 128, H), NPBF16)
        region[pad:] = h16[b, k_lo:k_hi]

        # feature-major [128, 8, 2304]
        hTr = np.ascontiguousarray(
            region.T.reshape(8, 128, RB * 128).transpose(1, 0, 2)
        )
        # keys-major [128, 18, 1024]
        haug = np.ascontiguousarray(
            region.reshape(RB, 128, H).transpose(1, 0, 2)
        )
        in_maps.append(
            {
                "hTr": hTr,
                "haug": haug,
                "Wr": Wr,
                "masks": masks_b if half == 0 else masks_i,
                "onesd": ones,
            }
        )
    return in_maps


def _assemble(results):
    out = np.empty((B, T, H), np.float32)
    for core in range(N_CORES):
        b, half = core // 2, core % 2
        out[b, half * QPC : (half + 1) * QPC] = (
            results[core]["out"].reshape(QPC, H)
        )
    return out


def kernel(h, W, T_hist):
    h = np.asarray(h, dtype=np.float32)
    W = np.asarray(W, dtype=np.float32)
    assert int(T_hist) == T_HIST
    assert h.shape == (B, T, H) and W.shape == (H, H)

    from concourse.bass_utils import run_bass_kernel_spmd

    nc = _build()
    in_maps = _prepare_in_maps(h, W)
    res = run_bass_kernel_spmd(nc, in_maps, core_ids=list(range(N_CORES)))
    return _assemble(res.results)


# revision 41
# speedup vs baseline: 1.0217x; 1.0047x over previous
"""Banded (Luong) attention TRN2 Bass kernel, 8-core SPMD, bf16 compute.

Problem: h [4, 4096, 1024] f32, W [1024, 1024] f32, T_hist=256.
  K = h @ W.T ; scores = (h @ K^T) / sqrt(H) with causal band
  (q - 255 <= k <= q); out = softmax(scores) @ h.

Sharding: data-parallel over batch (4) x sequence halves (2) -> 8 cores,
no cross-core communication. Each core handles 2048 queries; its key
region is 18 blocks of 128 (2 lead blocks for the band history,
zero-padded for the first half of each sequence).

Per-core algorithm (all matmuls bf16, fp32 PSUM accumulate; inputs are
converted to bf16 on the host, halving HBM traffic):
  warmup  dummy matmuls on zeroed SBUF hold the PE HAM clock gate at
          2.4 GHz while the first input DMAs land.
  proj    Q'T[m, q] = (h_q @ W)^T for all 2048 queries, 8-matmul
          accumulation groups of <=512 cols into 4-deep rotating PSUM,
          copied to SBUF bf16 on alternating DVE/ACT.
  ST      key-major: key block R scores against the 384 contiguous
          queries (q blocks R-2..R) that attend it -- one 8-matmul
          ap<=384 accumulation per R, LDWEIGHTS fully hidden. One
          shared additive [causal|full|upper] mask (DVE), exp with
          fused 1/32 scale (ACT) -> pt bf16.
  ctx     per query block: 6 matmuls of 512 cols accumulate P^T @ h;
          softmax denominators via DVE pre-sum of the 3 band slices +
          one ones-matmul into the st tile's spare column; normalize
          with per-partition 1/sum during the PSUM->SBUF copy
          (ACT/DVE split); DMA out per 512-col half.
  ctx_block(R-4) is issued after ST(R) so every softmax chain hides
  under PE work, keeping the matmul stream dense (no HAM re-throttle).
"""

import ml_dtypes
import numpy as np

import concourse.bass as bass
import concourse.mybir as mybir
import concourse.tile as tile
from concourse import bacc

B, T, H = 4, 4096, 1024
T_HIST = 256
N_CORES = 8
QB = 16            # 128-row query blocks per core
RB = QB + 2        # key-region blocks per core (2 lead blocks)
QPC = 2048         # queries per core
BF16 = mybir.dt.bfloat16
F32 = mybir.dt.float32
NEG = np.float32(-1e9)
INV_SQRT_H = 1.0 / 32.0
NPBF16 = ml_dtypes.bfloat16

_CACHE = {}


def _kernel_body(tc, out, hTr, haug, Wr, masks, onesd):
    nc = tc.nc

    with (
        tc.tile_pool(name="singles", bufs=1) as singles,
        tc.tile_pool(name="pt", bufs=6) as pt_pool,
        tc.tile_pool(name="ptsum", bufs=3) as ptsum_pool,
        tc.tile_pool(name="ctxs", bufs=3) as ctxs_pool,
        tc.tile_pool(name="recip", bufs=4) as recip_pool,
        tc.tile_pool(name="ps", bufs=4, space="PSUM") as ps_pool,
        tc.tile_pool(name="st", bufs=4, space="PSUM") as st_pool,
    ):
        # --- PE warmup: dummy matmuls on zeroed SBUF keep the HAM
        # activity window busy while the first input DMAs land ---
        warm = singles.tile([128, 256], BF16)
        nc.vector.memset(warm[:], 0.0)
        for w in range(60):
            wps = ps_pool.tile([128, 128], F32, tag="ps")
            nc.tensor.matmul(
                wps[:], warm[:, 0:128], warm[:, 128:256], start=True, stop=True
            )

        # --- resident inputs, DMA'd in exact consumption order: the W
        # slices feed proj groups mc=0..7 of tt=0 progressively. The
        # first hT pieces go on the ACT sequencer so their issue overlaps
        # the W issues on SYNC; masks/ha go on DVE (needed much later) ---
        W_sb = singles.tile([128, 8, H], BF16)            # 2.10 MiB
        hT_sb = singles.tile([128, 8, RB * 128], BF16)    # 4.72 MiB
        ha_sb = singles.tile([128, RB, H], BF16)          # 4.72 MiB
        qt = singles.tile([128, 8, QPC], BF16)            # 4.19 MiB
        mask_sb = singles.tile([128, 2, 384], F32)
        ones_sb = singles.tile([128, 1], BF16)

        nc.scalar.dma_start(hT_sb[:, :, 256:512], hTr[:, :, 256:512])
        nc.scalar.dma_start(hT_sb[:, :, 512:768], hTr[:, :, 512:768])
        for mc in range(8):
            nc.sync.dma_start(
                W_sb[:, :, mc * 128 : (mc + 1) * 128],
                Wr[:, :, mc * 128 : (mc + 1) * 128],
            )
        nc.sync.dma_start(hT_sb[:, :, 768:1280], hTr[:, :, 768:1280])
        nc.sync.dma_start(hT_sb[:, :, 1280:2304], hTr[:, :, 1280:2304])
        nc.sync.dma_start(hT_sb[:, :, 0:256], hTr[:, :, 0:256])
        nc.sync.dma_start(mask_sb[:], masks[:])
        nc.sync.dma_start(ones_sb[:], onesd[:])
        nc.sync.dma_start(ha_sb[:], haug[:])

        # --- projection: Q'T[m, q] for all 2048 queries. The first four
        # groups run at 256 cols so the PE can start on 0.76 MiB of input ---
        def proj_group(mc, q0, cw):
            ps = ps_pool.tile([128, cw], F32, tag="ps")
            for oc in range(8):
                nc.tensor.matmul(
                    ps[:],
                    W_sb[:, oc, mc * 128 : (mc + 1) * 128],
                    hT_sb[:, oc, 256 + q0 : 256 + q0 + cw],
                    start=(oc == 0),
                    stop=(oc == 7),
                )
            if mc % 2 == 0:
                nc.vector.tensor_copy(qt[:, mc, q0 : q0 + cw], ps[:])
            else:
                nc.scalar.copy(qt[:, mc, q0 : q0 + cw], ps[:])

        proj_group(0, 0, 256)
        proj_group(1, 0, 256)
        proj_group(0, 256, 256)
        proj_group(1, 256, 256)
        for mc in range(2, 8):
            proj_group(mc, 0, 512)
        for tt in range(1, 4):
            for mc in range(8):
                proj_group(mc, tt * 512, 512)

        # --- attention, key-major: key block R scores against the 384
        # contiguous queries (q blocks R-2..R) that attend it, one 8-matmul
        # accumulation of ap<=384 per R (LDWEIGHTS fully hidden). Column c
        # of st_R belongs to q block i = R-2 + c//128 at band position
        # j = R - i, so the additive mask is the same [caus|full|su]
        # pattern for every interior R. ---
        st_tiles = {}   # per key block R
        pt_tiles = {}

        def ST(R):
            st = st_pool.tile([128, 388], F32, tag="st")
            st_tiles[R] = st
            lo = max(0, 2 - R) * 128
            hi = (min(15, R) - R + 3) * 128
            qlo = (R - 2) * 128 + lo
            for mc in range(8):
                nc.tensor.matmul(
                    st[:, lo:hi],
                    hT_sb[:, mc, R * 128 : (R + 1) * 128],
                    qt[:, mc, qlo : qlo + hi - lo],
                    start=(mc == 0), stop=(mc == 7),
                )
            msel = 1 if R <= 1 else 0    # lead blocks get the boundary mask
            nc.vector.tensor_add(
                st[:, lo:hi], st[:, lo:hi], mask_sb[:, msel, lo:hi]
            )
            pt = pt_pool.tile([128, 384], BF16, tag="pt")
            nc.scalar.activation(
                pt[:, lo:hi], st[:, lo:hi], mybir.ActivationFunctionType.Exp,
                scale=INV_SQRT_H,
            )
            pt_tiles[R] = pt

        def ctx_block(i):
            # band slice j of q block i lives in pt_{i+j} cols (2-j)*128
            sl = [
                pt_tiles[i + j][:, (2 - j) * 128 : (3 - j) * 128]
                for j in range(3)
            ]
            psm = ptsum_pool.tile([128, 128], BF16, tag="ptsum")
            nc.vector.tensor_add(psm[:], sl[0], sl[1])
            nc.vector.tensor_add(psm[:], psm[:], sl[2])
            sums = st_tiles[i + 2]
            nc.tensor.matmul(
                sums[:, 384:385], psm[:], ones_sb[:], start=True, stop=True
            )
            ca = ps_pool.tile([128, 512], F32, tag="ps")
            cb = ps_pool.tile([128, 512], F32, tag="ps")
            for j in range(3):
                nc.tensor.matmul(
                    ca[:], sl[j], ha_sb[:, i + j, 0:512],
                    start=(j == 0), stop=(j == 2),
                )
                nc.tensor.matmul(
                    cb[:], sl[j], ha_sb[:, i + j, 512:1024],
                    start=(j == 0), stop=(j == 2),
                )
            recip = recip_pool.tile([128, 1], F32)
            nc.vector.reciprocal(recip[:], sums[:, 384:385])
            ctxs = ctxs_pool.tile([128, 1024], F32)
            nc.scalar.mul(ctxs[:, 0:512], ca[:], mul=recip[:])
            nc.sync.dma_start(out[i][:, 0:512], ctxs[:, 0:512])
            nc.vector.tensor_scalar_mul(ctxs[:, 512:1024], cb[:], recip[:])
            nc.sync.dma_start(out[i][:, 512:1024], ctxs[:, 512:1024])

        for R in range(RB):
            ST(R)
            if R >= 4:
                ctx_block(R - 4)
        for i in range(14, 16):
            ctx_block(i)


def _build():
    if "nc" in _CACHE:
        return _CACHE["nc"]
    nc = bacc.Bacc(
        "TRN2", target_bir_lowering=False, debug=False, num_devices=N_CORES
    )
    hTr = nc.dram_tensor("hTr", [128, 8, RB * 128], BF16, kind="ExternalInput").ap()
    haug = nc.dram_tensor("haug", [128, RB, H], BF16, kind="ExternalInput").ap()
    Wr = nc.dram_tensor("Wr", [128, 8, H], BF16, kind="ExternalInput").ap()
    masks = nc.dram_tensor("masks", [128, 2, 384], F32, kind="ExternalInput").ap()
    onesd = nc.dram_tensor("onesd", [128, 1], BF16, kind="ExternalInput").ap()
    out = nc.dram_tensor("out", [QB, 128, H], F32, kind="ExternalOutput").ap()
    with tile.TileContext(nc) as tc:
        _kernel_body(tc, out, hTr, haug, Wr, masks, onesd)
    nc.compile()
    _CACHE["nc"] = nc
    return nc


def _host_masks():
    # column c of an st_R tile: band position j = 2 - c//128
    # j=2 -> causal (kk <= qi), j=1 -> full, j=0 -> strict upper (kk > qi)
    kk = np.arange(128, dtype=np.int64)[:, None]
    qi = np.arange(128, dtype=np.int64)[None, :]
    su = np.where(kk > qi, np.float32(0.0), NEG).astype(np.float32)
    caus = np.where(kk <= qi, np.float32(0.0), NEG).astype(np.float32)
    interior = np.empty((128, 384), np.float32)
    interior[:, 0:128] = caus
    interior[:, 128:256] = 0.0
    interior[:, 256:384] = su
    boundary = np.full((128, 384), NEG, np.float32)
    return interior, boundary


def _prepare_in_maps(h, W):
    interior, boundary = _host_masks()
    masks_b = np.ascontiguousarray(np.stack([interior, boundary], axis=1))
    masks_i = np.ascontiguousarray(np.stack([interior, interior], axis=1))
    h16 = h.astype(NPBF16)
    W16 = W.astype(NPBF16)
    Wr = np.ascontiguousarray(W16.reshape(8, 128, H).transpose(1, 0, 2))
    ones = np.ones((128, 1), NPBF16)
    in_maps = []
    for core in range(N_CORES):
        b, half = core // 2, core % 2
        k_lo = half * QPC - 256            # region global key start
        pad = max(0, -k_lo)                # 256 for half 0, else 0
        k_lo = max(0, k_lo)
        k_hi = half * QPC + QPC

        # region keys-major [2304, 1024] bf16, zero lead pad
        region = np.zeros((RB * 128, H), NPBF16)
        region[pad:] = h16[b, k_lo:k_hi]

        # feature-major [128, 8, 2304]
        hTr = np.ascontiguousarray(
            region.T.reshape(8, 128, RB * 128).transpose(1, 0, 2)
        )
        # keys-major [128, 18, 1024]
        haug = np.ascontiguousarray(
            region.reshape(RB, 128, H).transpose(1, 0, 2)
        )
        in_maps.append(
            {
                "hTr": hTr,
                "haug": haug,
                "Wr": Wr,
                "masks": masks_b if half == 0 else masks_i,
                "onesd": ones,
            }
        )
    return in_maps


def _assemble(results):
    out = np.empty((B, T, H), np.float32)
    for core in range(N_CORES):
        b, half = core // 2, core % 2
        out[b, half * QPC : (half + 1) * QPC] = (
            results[core]["out"].reshape(QPC, H)
        )
    return out


def kernel(h, W, T_hist):
    h = np.asarray(h, dtype=np.float32)
    W = np.asarray(W, dtype=np.float32)
    assert int(T_hist) == T_HIST
    assert h.shape == (B, T, H) and W.shape == (H, H)

    from concourse.bass_utils import run_bass_kernel_spmd

    nc = _build()
    in_maps = _prepare_in_maps(h, W)
    res = run_bass_kernel_spmd(nc, in_maps, core_ids=list(range(N_CORES)))
    return _assemble(res.results)
